# revision 5
# baseline (speedup 1.0000x reference)
"""Causal single-head attention (B=4, S=4096, E=1024, D=128) on 8 TRN2 NeuronCores.

Sharding: core = (batch b, query-group g) with b = core//2, g = core%2.
Quad m = abs tiles {4m..4m+3}; member g=0 owns {4m, 4m+3}, g=1 owns
{4m+1, 4m+2} — this split balances the causal diagonal work exactly 5/5
per quad, so the diagonal can be computed with 6 exact N=128 MMs
[s0q1, s2q1, s0q2, s1q2, s2q2, s3q2] (one wasted MM per quad vs 3 for a
masked full quad; mask input absorbs the per-member asymmetry).  Local key
slot order per quad: [own t0, own t1, other t0, other t1] via host-side
x packing; attention is permutation-invariant over keys so only the mask
and the output row mapping see the permutation.

Per-core device program (bf16 matmuls, fp32 accumulate):
  Q_T = Wq.T @ xq.T   [d=128, 2048]
  K_T = Wk.T @ x.T    [d=128, 4096]
  V   = x @ Wv        [4096, 128] as 32 partition-tiles + ones column so
                      the AV matmul accumulates the softmax denominator.
  per pair m: diag supertile [128, 768] (6 N=128 MMs) + quads j<m as
      [128, 1024] supertiles (4 N=256 MMs); exp on ACT (no row-max:
      scores ~N(0,1)); diag multiplied by the 0/1/tri mask on DVE.
  AV: q1 chain 4m+2 MMs, q2 chain 4m+4 MMs of N=129 in separate PSUM
      banks; epilogue divides by the accumulated denominator.

Schedule: x packed as contiguous 256-col blocks; pairs processed in order
1..7,0 so the closing AV chain is the smallest; pair 0's scores are
emitted during pair 7.  The startup is DMA-ramp-gated (first x data
~13.5-14us regardless of trigger scheme): a 16-MM warmup on a memset
tile (gpsimd memset, so the first LDW issues at ~7.5us) keeps the PE
busy through the HAM window — 8 cold MMs flip the clock at ~11.6us, the
rest bridge to data arrival, since any warm-state gap >1.7us re-throttles.
Block 0's work is consumed in x-piece ARRIVAL order (K-h0, Q, V0, V1,
then K-h1, V2, V3) so a late second half never stalls the PE, and block
1's x is prefetched on the scalar queue behind wv/wq.
"""

import sys
import types

import numpy as np
import ml_dtypes

B, S, E, D = 4, 4096, 1024, 128
N_CORES = 8
NPAIR = 8
PAIR_W = 256
SQ = S // 2
SCALE = float(D) ** -0.5
N_WARMUP = 16   # bridge 8.2us (first possible MM) to ~13.9us (first x data,
                # run-variable +-1.5us): 8 cold MMs x 427ns flip the HAM at
                # ~11.6us, the rest run warm at 213ns.  Too-short warmup
                # leaves a >1.7us gap that re-throttles the warm clock (MID
                # window at 2.4GHz), which cost ~2-5us in measurements.
ORDER = [1, 2, 3, 4, 5, 6, 7, 0]
OWN_T = {0: (0, 3), 1: (1, 2)}
SLOTMAP = {0: (0, 3, 1, 2), 1: (1, 2, 0, 3)}  # slot -> rel tile within quad

_cache = {}


def _install_ntff_shim():
    if "antenv.axon_hooks" in sys.modules:
        return
    try:
        from trn_agent_boot.trn_boot import _ntff_profile_via_ctypes

        hook = _ntff_profile_via_ctypes("/opt/axon/libaxon_pjrt.so")
    except Exception:
        hook = None
    mod = types.ModuleType("antenv.axon_hooks")
    mod.get_axon_ntff_profile_hook = lambda: hook
    mod.set_axon_ntff_profile_hook = lambda h: None
    sys.modules["antenv.axon_hooks"] = mod


def build_nc():
    from contextlib import ExitStack

    import concourse.tile as tile
    from concourse import bacc, mybir
    from concourse.bass import ds, ts

    f32 = mybir.dt.float32
    bf16 = mybir.dt.bfloat16

    nc = bacc.Bacc("TRN2", target_bir_lowering=False, debug=False)
    xt = nc.dram_tensor("xt", [128, 16, 8, 256], bf16, kind="ExternalInput").ap()
    w_all = nc.dram_tensor("w_all", [128, 3, 8, D], bf16, kind="ExternalInput").ap()
    mask = nc.dram_tensor("mask", [128, 768], bf16, kind="ExternalInput").ap()
    out = nc.dram_tensor("out", [SQ, D], f32, kind="ExternalOutput").ap()

    with tile.TileContext(nc) as tc, ExitStack() as ctx:
        consts = ctx.enter_context(tc.tile_pool(name="consts", bufs=1))
        big = ctx.enter_context(tc.tile_pool(name="big", bufs=1))
        x_pool = ctx.enter_context(tc.tile_pool(name="xs", bufs=3))
        pt_pool = ctx.enter_context(tc.tile_pool(name="pt", bufs=20))
        osb_pool = ctx.enter_context(tc.tile_pool(name="osb", bufs=4))
        rec_pool = ctx.enter_context(tc.tile_pool(name="rec", bufs=4))
        sp = ctx.enter_context(tc.tile_pool(name="sp", bufs=2, space="PSUM"))
        avp = ctx.enter_context(tc.tile_pool(name="avp", bufs=2, space="PSUM"))
        vp = ctx.enter_context(tc.tile_pool(name="vp", bufs=2, space="PSUM"))

        # PE warmup: dense matmuls on a memset tile with no DMA dependency,
        # so the HAM un-throttles while the first input DMAs fly.
        warm_sb = consts.tile([128, 512], bf16)
        nc.gpsimd.memset(warm_sb[:], 1.0)
        warm_ps = sp.tile([128, 512], f32, tag="sps")
        for _ in range(N_WARMUP):
            nc.tensor.matmul(
                warm_ps[:], lhsT=warm_sb[:, 0:128], rhs=warm_sb[:],
                start=True, stop=True,
            )

        w_sb = consts.tile([128, 3, 8, D], bf16)
        wq_sb, wk_sb, wv_sb = w_sb[:, 0], w_sb[:, 1], w_sb[:, 2]
        # first-needed data spread over four queues so the triggers all fire
        # at ~7.2us instead of serializing on sync
        nc.sync.dma_start(w_sb[:, 1], w_all[:, 1])  # wk: K0 leads
        mask_sb = consts.tile([128, 768], bf16)

        qt_sb = big.tile([128, SQ], bf16)
        kt_sb = big.tile([128, S], bf16)
        v_sb = big.tile([128, 32, D + 1], bf16)
        nc.vector.memset(v_sb[:, :, D : D + 1], 1.0)

        def x_fetch(b, nsplit=1, engs=None):
            x_t = x_pool.tile([128, 2, 8, 256], bf16, tag="xs")
            ec = 8 // nsplit
            for h in range(2):
                for si in range(nsplit):
                    eng = engs[h * nsplit + si] if engs else nc.sync
                    eng.dma_start(
                        x_t[:, h, ds(si * ec, ec), :],
                        xt[:, 2 * b + h, ds(si * ec, ec), :],
                    )
            return x_t

        def kv_block(b, nsplit=1, engs=None, defer_q=False, x_pre=None):
            """512 key columns = xt sub-blocks 2b (own tiles: slots 0,1),
            2b+1 (other tiles: slots 2,3).  Q projection for pair b reads
            cols 0:256 of the same x tile."""
            x_t = x_pre if x_pre is not None else x_fetch(b, nsplit, engs)
            ps = sp.tile([128, 1024], f32, tag="sps")

            def k_half(h):
                # h0 and h1 chains write disjoint 256-col regions of the same
                # PSUM tile; h1's start=True clears has_written bank-wide,
                # which is safe because h0's chain has already stopped (any
                # interleaved work uses other PSUM tiles).
                for e in range(8):
                    nc.tensor.matmul(
                        ps[:, ds(h * 256, 256)],
                        lhsT=wk_sb[:, e, :],
                        rhs=x_t[:, h, e, :],
                        start=(e == 0),
                        stop=(e == 7),
                    )
                nc.vector.tensor_copy(
                    kt_sb[:, ds(b * 512 + h * 256, 256)], ps[:, ds(h * 256, 256)]
                )

            def q_emit():
                qps = sp.tile([128, 1024], f32, tag="sps")
                for e in range(8):
                    nc.tensor.matmul(
                        qps[:, 0:256],
                        lhsT=wq_sb[:, e, :],
                        rhs=x_t[:, 0, e, :],
                        start=(e == 0),
                        stop=(e == 7),
                    )
                nc.vector.tensor_copy(qt_sb[:, ds(b * 256, 256)], qps[:, 0:256])

            def v_emit(st):
                psv = vp.tile([128, D], f32, tag="vps")
                for e in range(8):
                    nc.tensor.matmul(
                        psv[:],
                        lhsT=x_t[:, st // 2, e, ds((st % 2) * 128, 128)],
                        rhs=wv_sb[:, e, :],
                        start=(e == 0),
                        stop=(e == 7),
                    )
                nc.vector.tensor_copy(v_sb[:, 4 * b + st, 0:D], psv[:])

            if defer_q:
                # startup block: consume x pieces in arrival order — all
                # h0-only work (K-h0, Q, V0, V1) before anything touching h1,
                # so a late h1 piece does not stall the PE
                k_half(0)
                chunks = [lambda: v_emit(0), lambda: v_emit(1),
                          lambda: k_half(1), lambda: v_emit(2),
                          lambda: v_emit(3)]
                return chunks, q_emit
            k_half(0)
            k_half(1)
            q_emit()
            return [lambda st=st: v_emit(st) for st in range(4)]

        def kslot(j, s):
            return kt_sb[:, ds(j * 512 + s * 128, 128)]

        def attn_scores(m):
            """Diag supertile (6 N=128 MMs, masked) first so its exp+mask
            latency hides behind the full-quad score MMs; then full quads."""
            pts = {}
            chunks = []

            def emit_diag():
                sps = sp.tile([128, 1024], f32, tag="sps")
                q1 = qt_sb[:, ds(m * PAIR_W, 128)]
                q2 = qt_sb[:, ds(m * PAIR_W + 128, 128)]
                plan = [
                    (kslot(m, 0), q1), (kslot(m, 2), q1),
                    (kslot(m, 0), q2), (kslot(m, 1), q2),
                    (kslot(m, 2), q2), (kslot(m, 3), q2),
                ]
                for k, (lhsT, rhs) in enumerate(plan):
                    nc.tensor.matmul(
                        sps[:, ds(k * 128, 128)], lhsT=lhsT, rhs=rhs,
                        start=True, stop=True,
                    )
                ptd = pt_pool.tile([128, 1024], bf16)
                nc.scalar.activation(
                    ptd[:, 0:768], sps[:, 0:768],
                    func=mybir.ActivationFunctionType.Exp, scale=SCALE,
                )
                nc.vector.tensor_mul(ptd[:, 0:768], ptd[:, 0:768], mask_sb[:])
                pts["diag"] = ptd

            chunks.append(emit_diag)
            for j in range(m):
                def emit_full(j=j):
                    sps = sp.tile([128, 1024], f32, tag="sps")
                    for k in range(4):
                        nc.tensor.matmul(
                            sps[:, ds(k * PAIR_W, PAIR_W)],
                            lhsT=kslot(j, k),
                            rhs=qt_sb[:, ds(m * PAIR_W, PAIR_W)],
                            start=True,
                            stop=True,
                        )
                    pt = pt_pool.tile([128, 1024], bf16)
                    nc.scalar.activation(
                        pt[:], sps[:],
                        func=mybir.ActivationFunctionType.Exp, scale=SCALE,
                    )
                    pts[j] = pt
                chunks.append(emit_full)
            return pts, chunks

        def attn_av_chunks(m, pts):
            ops = [
                avp.tile([128, D + 1], f32, tag="avs", name=f"av_{m}_{c}")
                for c in range(2)
            ]
            nmm = [4 * m + 2, 4 * m + 4]

            def fin(c):
                rc = rec_pool.tile([128, 1], f32)
                nc.vector.reciprocal(rc[:], ops[c][:, D : D + 1])
                osb = osb_pool.tile([128, D], f32)
                nc.vector.tensor_scalar_mul(osb[:], ops[c][:, 0:D], rc[:])
                # the very last output tile issues on the (idle) scalar queue
                # so the two closing DMAs overlap instead of serializing
                eng = nc.scalar if (m == 0 and c == 1) else nc.sync
                eng.dma_start(out[ts(2 * m + c, 128), :], osb[:])

            # accumulation plans: full quads (pt slot k -> v slot (j, k)),
            # then the diag contributions from the masked supertile.
            plans = {0: [], 1: []}
            for j in range(m):
                for k in range(4):
                    for c in range(2):
                        plans[c].append(
                            (pts[j][:, ds(k * PAIR_W + c * 128, 128)],
                             v_sb[:, 4 * j + k, :])
                        )
            ptd = pts["diag"]
            plans[0].append((ptd[:, ds(0, 128)], v_sb[:, 4 * m + 0, :]))
            plans[0].append((ptd[:, ds(128, 128)], v_sb[:, 4 * m + 2, :]))
            plans[1].append((ptd[:, ds(256, 128)], v_sb[:, 4 * m + 0, :]))
            plans[1].append((ptd[:, ds(384, 128)], v_sb[:, 4 * m + 1, :]))
            plans[1].append((ptd[:, ds(512, 128)], v_sb[:, 4 * m + 2, :]))
            plans[1].append((ptd[:, ds(640, 128)], v_sb[:, 4 * m + 3, :]))
            assert len(plans[0]) == nmm[0] and len(plans[1]) == nmm[1]

            chunks = []
            for t0 in range(0, nmm[1], 4):
                for c in range(2):
                    lo, hi = t0, min(t0 + 4, nmm[c])
                    if lo >= hi:
                        continue
                    def emit(c=c, lo=lo, hi=hi, last=(hi == nmm[c])):
                        for t in range(lo, hi):
                            pt_ap, v_ap = plans[c][t]
                            nc.tensor.matmul(
                                ops[c][:], lhsT=pt_ap, rhs=v_ap,
                                start=(t == 0), stop=(t == nmm[c] - 1),
                            )
                        if last:
                            fin(c)
                    chunks.append(emit)
            return chunks

        # Software pipeline over pairs in ORDER, as in v1: block 0's four
        # x DMAs ride four different queues so they all fire at ~7.2us.
        v0, q0 = kv_block(0, nsplit=2, defer_q=True)
        nc.scalar.dma_start(w_sb[:, 2], w_all[:, 2])  # wv
        nc.scalar.dma_start(w_sb[:, 0], w_all[:, 0])  # wq
        # prefetch block 1's x on the scalar queue so the PE has continuous
        # work through the DMA-starved 12-16us window (a warm-state PE gap
        # >1.7us re-throttles the clock)
        x1_pre = x_fetch(1, nsplit=2, engs=[nc.scalar, nc.scalar, nc.scalar, nc.scalar])
        nc.scalar.dma_start(mask_sb[:], mask)
        q0()
        for ch in v0:
            ch()
        prev_av = []
        pts0 = None
        s0_chunks = None
        for i, m in enumerate(ORDER):
            v_chunks = []
            if i == 0:
                v_chunks += kv_block(1, x_pre=x1_pre)
            elif m != 0:
                v_chunks += kv_block(m)
            if m == 7:
                # Pair 0's scores emitted now so the closing AV(0) chain has
                # no activation dependency left.
                pts0, s0_chunks = attn_scores(0)
                for sc in s0_chunks:
                    sc()
            if m == 0:
                pts, s_chunks = pts0, []
            else:
                pts, s_chunks = attn_scores(m)
            fillers = v_chunks + prev_av
            na, ns = len(fillers), len(s_chunks)
            ai = 0
            for si, sc in enumerate(s_chunks):
                sc()
                want = (si + 1) * na // ns
                while ai < want:
                    fillers[ai]()
                    ai += 1
            while ai < na:
                fillers[ai]()
                ai += 1
            prev_av = attn_av_chunks(m, pts)
        for ch in prev_av:
            ch()

    nc.compile()
    return nc


def _qrows(g: int) -> np.ndarray:
    rows = np.empty(SQ, np.int64)
    o = OWN_T[g]
    for L in range(16):
        m, c = divmod(L, 2)
        a = 4 * m + o[c]
        rows[L * 128 : (L + 1) * 128] = np.arange(a * 128, (a + 1) * 128)
    return rows


def _mask(g: int) -> np.ndarray:
    tri = (np.arange(128)[:, None] <= np.arange(128)[None, :]).astype(np.float32)
    one = np.ones((128, 128), np.float32)
    zero = np.zeros((128, 128), np.float32)
    if g == 0:
        blocks = [tri, zero, one, tri, one, one]
    else:
        blocks = [tri, one, one, tri, one, zero]
    return np.concatenate(blocks, axis=1).astype(ml_dtypes.bfloat16)


def _pack_w(w: np.ndarray) -> np.ndarray:
    bf = ml_dtypes.bfloat16
    return np.ascontiguousarray(
        np.asarray(w, np.float32).reshape(8, 128, D).transpose(1, 0, 2)
    ).astype(bf)


def build_in_maps(x, Wq, Wk, Wv):
    bf = ml_dtypes.bfloat16
    x16 = np.asarray(x, np.float32).astype(bf)
    w_all = np.ascontiguousarray(
        np.stack([_pack_w(Wq), _pack_w(Wk), _pack_w(Wv)], axis=1)
    )
    masks = {g: _mask(g) for g in (0, 1)}

    in_maps = []
    for core in range(N_CORES):
        b, g = divmod(core, 2)
        sm = SLOTMAP[g]
        xTb = np.ascontiguousarray(x16[b].T)  # [E, S] bf16
        xr = xTb.reshape(8, 128, 32, 128)     # [e, p, tile, col]
        xt = np.empty((128, 16, 8, 256), bf)
        for m in range(NPAIR):
            for sub in range(2):
                for half in range(2):
                    a = 4 * m + sm[2 * sub + half]
                    xt[:, 2 * m + sub, :, 128 * half : 128 * (half + 1)] = (
                        xr[:, :, a, :].transpose(1, 0, 2)
                    )
        in_maps.append(
            {
                "xt": np.ascontiguousarray(xt),
                "w_all": w_all,
                "mask": masks[g],
            }
        )
    return in_maps


def kernel(x, Wq, Wk, Wv):
    _install_ntff_shim()
    from concourse.bass_utils import run_bass_kernel_spmd

    if "nc" not in _cache:
        _cache["nc"] = build_nc()
    nc = _cache["nc"]

    in_maps = build_in_maps(x, Wq, Wk, Wv)
    res = run_bass_kernel_spmd(nc, in_maps, core_ids=list(range(N_CORES)))
    qrows = {g: _qrows(g) for g in (0, 1)}
    out = np.empty((B, S, D), np.float32)
    for core in range(N_CORES):
        b, g = divmod(core, 2)
        out[b][qrows[g]] = res.results[core]["out"]
    return out
